# revision 3
# baseline (speedup 1.0000x reference)
"""Trainium2 Bass kernel for: out = A @ dequant_int4(weight, weight_scale) + bias.

Problem shapes (fp32 A, packed-int4 weight):
    A            [8192, 4096] f32
    weight       [2048, 11008] u8   (two int4 nibbles per byte along K;
                                     row 2i = low nibble, row 2i+1 = high nibble)
    weight_scale [128, 11008] f32   (per-group scale, group_size=32 along K)
    bias         [11008] f32
    out          [8192, 11008] f32

Sharding: tensor-parallel along out_features N across 8 NeuronCores.
Each core gets the full A, a 1376-wide column slice of weight/scale/bias and
computes its [8192, 1376] output slice; the host concatenates slices.

Host prep (layout/dtype only): A is cast to bf16 (the on-device matmul runs
in bf16 either way; previously a cast-DMA did this), packed int4 bytes are
unpacked to one nibble per u8 byte in natural k-order [4096, N], scales are
repeated 32x to per-k rows (bf16), bias replicated to [128, ns].  All
arithmetic (bias-8 shift, scale multiply, GEMM, bias add) happens on device.

Per-core kernel strategy:
  - Dequant once into a resident SBUF buffer wsb[p, kb, n] (natural k-order:
    k = 128*kb + p) with two DVE ops per element: (nib - 8) -> bf16, * scale.
  - Per 128-row chunk of A: one xbar DMA-transpose (DRAM->SBUF, HW-validated
    ~261 GB/s) produces at[p, kb, m] = A[m0+m, 128*kb+p] directly.  The PE
    runs pure matmuls: 3 n-chunks x 32 k-blocks accumulating in PSUM.  Bias
    is added during the PSUM->SBUF eviction on the DVE.
  - A-transposes ride the SP HWDGE ring (nc.sync); all other DMAs ride the
    ACT ring (nc.scalar) so the two FIFOs drain in parallel.
"""

import numpy as np
import ml_dtypes

import concourse.bacc as bacc
import concourse.tile as tile
from concourse import mybir
from concourse.bass_utils import run_bass_kernel_spmd

M, K, N = 8192, 4096, 11008
NCORES = 8
NS = N // NCORES  # 1376 out-features per core
P = 128
NKB = K // P      # 32 k-blocks
SB = 4            # k-blocks per dequant super-DMA
N_CHUNKS = [(0, 512), (512, 512), (1024, 352)]


def build_nc(m=M, ns=NS, reps=1, debug=False):
    """Build the per-core Bass program (identical on all cores)."""
    mch = m // P

    nc = bacc.Bacc(None, target_bir_lowering=False, debug=debug)
    A = nc.dram_tensor("A", [m, K], mybir.dt.bfloat16, kind="ExternalInput")
    NIB = nc.dram_tensor("nib", [K, ns], mybir.dt.uint8, kind="ExternalInput")
    SREP = nc.dram_tensor("srep", [K, ns], mybir.dt.bfloat16, kind="ExternalInput")
    BIAS = nc.dram_tensor("bias", [P, ns], mybir.dt.float32, kind="ExternalInput")
    OUT = nc.dram_tensor("out", [m, ns], mybir.dt.float32, kind="ExternalOutput")

    with tile.TileContext(nc) as tc:
        with (
            tc.tile_pool(name="singles", bufs=1) as singles,
            tc.tile_pool(name="wpool", bufs=1) as wpool,
            tc.tile_pool(name="dq", bufs=2) as dq,
            tc.tile_pool(name="atpool", bufs=3) as atpool,
            tc.tile_pool(name="opool", bufs=3) as opool,
            tc.tile_pool(name="psum_o", bufs=4, space="PSUM") as psum_o,
        ):
            bias_t = singles.tile([P, ns], mybir.dt.float32)
            nc.scalar.dma_start(out=bias_t, in_=BIAS[:, :])

            # ---- one-shot dequant into resident SBUF, natural k-order ----
            wsb = wpool.tile([P, NKB, ns], mybir.dt.bfloat16)
            for sb in range(NKB // SB):
                pk = dq.tile([P, SB, ns], mybir.dt.uint8, tag="pk")
                st = dq.tile([P, SB, ns], mybir.dt.bfloat16, tag="st")
                rows = slice(sb * SB * P, (sb + 1) * SB * P)
                nc.scalar.dma_start(
                    out=pk, in_=NIB[rows, :].rearrange("(b p) n -> p b n", p=P))
                nc.scalar.dma_start(
                    out=st, in_=SREP[rows, :].rearrange("(b p) n -> p b n", p=P))
                for j in range(SB):
                    kb = sb * SB + j
                    v = dq.tile([P, ns], mybir.dt.bfloat16, tag="v")
                    nc.vector.tensor_scalar(
                        out=v, in0=pk[:, j, :], scalar1=8, scalar2=None,
                        op0=mybir.AluOpType.subtract)
                    nc.vector.tensor_tensor(
                        out=wsb[:, kb, :], in0=v, in1=st[:, j, :],
                        op=mybir.AluOpType.mult)

            # ---- main loop over 128-row chunks of A ----
            for _rep in range(reps):
                for mc in range(mch):
                    at = atpool.tile([P, NKB, P], mybir.dt.bfloat16)
                    nc.sync.dma_start_transpose(
                        at[:, :, :], A[mc * P:(mc + 1) * P, :])

                    o_sb = opool.tile([P, ns], mybir.dt.float32)
                    for (n0, nch) in N_CHUNKS:
                        po = psum_o.tile([P, 512], mybir.dt.float32, tag="po")
                        for kb in range(NKB):
                            nc.tensor.matmul(
                                po[:, :nch], lhsT=at[:, kb, :],
                                rhs=wsb[:, kb, n0:n0 + nch],
                                start=(kb == 0), stop=(kb == NKB - 1))
                        nc.vector.tensor_tensor(
                            out=o_sb[:, n0:n0 + nch], in0=po[:, :nch],
                            in1=bias_t[:, n0:n0 + nch], op=mybir.AluOpType.add)
                    nc.scalar.dma_start(out=OUT[mc * P:(mc + 1) * P, :], in_=o_sb)

    nc.finalize()
    return nc


_NC_CACHE = {}


def _get_nc():
    if "nc" not in _NC_CACHE:
        _NC_CACHE["nc"] = build_nc()
    return _NC_CACHE["nc"]


def shard_inputs(A, weight, weight_scale, bias):
    A = np.asarray(A, dtype=np.float32).astype(ml_dtypes.bfloat16)
    wq = np.asarray(weight, dtype=np.uint8)
    ws = np.asarray(weight_scale, dtype=np.float32)
    bs = np.asarray(bias, dtype=np.float32)

    # unpack nibbles to natural k-order: row 2i = low nibble, row 2i+1 = high
    nib = np.empty((K, N), dtype=np.uint8)
    nib[0::2] = wq & 15
    nib[1::2] = wq >> 4
    srep = np.repeat(ws, K // ws.shape[0], axis=0).astype(ml_dtypes.bfloat16)

    in_maps = []
    for c in range(NCORES):
        sl = slice(c * NS, (c + 1) * NS)
        in_maps.append({
            "A": A,
            "nib": np.ascontiguousarray(nib[:, sl]),
            "srep": np.ascontiguousarray(srep[:, sl]),
            # partition-replicated so the device DMA is a plain 2D copy
            "bias": np.ascontiguousarray(np.broadcast_to(bs[sl], (P, NS))),
        })
    return in_maps


def run(inputs, trace=False, **kw):
    nc = _get_nc()
    in_maps = shard_inputs(**inputs)
    res = run_bass_kernel_spmd(nc, in_maps, core_ids=list(range(NCORES)), trace=trace, **kw)
    out = np.concatenate([res.results[c]["out"] for c in range(NCORES)], axis=1)
    return out, res


def kernel(A, weight, weight_scale, bias):
    out, _ = run(dict(A=A, weight=weight, weight_scale=weight_scale, bias=bias))
    return out


# revision 4
# speedup vs baseline: 1.2757x; 1.2757x over previous
"""Trainium2 Bass kernel for: out = A @ dequant_int4(weight, weight_scale) + bias.

Problem shapes (fp32 A, packed-int4 weight):
    A            [8192, 4096] f32
    weight       [2048, 11008] u8   (two int4 nibbles per byte along K;
                                     row 2i = low nibble, row 2i+1 = high nibble)
    weight_scale [128, 11008] f32   (per-group scale, group_size=32 along K)
    bias         [11008] f32
    out          [8192, 11008] f32

Sharding: tensor-parallel along out_features N across 8 NeuronCores.
Each core gets the full A, a 1376-wide column slice of weight/scale/bias and
computes its [8192, 1376] output slice; the host concatenates slices.

Host prep (layout/dtype only): A is cast to bf16 and re-laid-out into
k-major transposed tiles atb[mc, p, kb, m] = A[128*mc + m, 128*kb + p] so
each 128-row chunk's A^T tile image is one contiguous 1 MB DMA; packed int4
bytes are unpacked to one nibble per u8 byte in natural k-order [4096, N];
scales are repeated 32x to per-k rows (bf16); bias replicated to [128, ns].
All arithmetic (bias-8 shift, scale multiply, GEMM, bias add) happens on
device.

Per-core kernel strategy:
  - Dequant once into a resident SBUF buffer wsb[p, kb, n] (natural k-order:
    k = 128*kb + p) with two DVE ops per element: (nib - 8) -> bf16, * scale.
  - Per 128-row chunk of A: one contiguous DMA loads at[p, kb, m]; the PE
    runs pure matmuls: 3 n-chunks x 32 k-blocks accumulating in PSUM.  Bias
    is added during the PSUM->SBUF eviction on the DVE.
  - A-tile loads ride the SP HWDGE ring (nc.sync); all other DMAs ride the
    ACT ring (nc.scalar) so the two FIFOs drain in parallel.
"""

import numpy as np
import ml_dtypes

import concourse.bacc as bacc
import concourse.tile as tile
from concourse import mybir
from concourse.bass_utils import run_bass_kernel_spmd

M, K, N = 8192, 4096, 11008
NCORES = 8
NS = N // NCORES  # 1376 out-features per core
P = 128
NKB = K // P      # 32 k-blocks
MCH = M // P      # 64 m-chunks
SB = 4            # k-blocks per dequant super-DMA
N_CHUNKS = [(0, 512), (512, 512), (1024, 352)]


def build_nc(m=M, ns=NS, reps=1, debug=False):
    """Build the per-core Bass program (identical on all cores)."""
    mch = m // P

    nc = bacc.Bacc(None, target_bir_lowering=False, debug=debug)
    ATB = nc.dram_tensor("atb", [mch, P, NKB, P], mybir.dt.bfloat16, kind="ExternalInput")
    NIB = nc.dram_tensor("nib", [K, ns], mybir.dt.uint8, kind="ExternalInput")
    SREP = nc.dram_tensor("srep", [K, ns], mybir.dt.bfloat16, kind="ExternalInput")
    BIAS = nc.dram_tensor("bias", [P, ns], mybir.dt.float32, kind="ExternalInput")
    OUT = nc.dram_tensor("out", [m, ns], mybir.dt.float32, kind="ExternalOutput")

    with tile.TileContext(nc) as tc:
        with (
            tc.tile_pool(name="singles", bufs=1) as singles,
            tc.tile_pool(name="wpool", bufs=1) as wpool,
            tc.tile_pool(name="dq", bufs=2) as dq,
            tc.tile_pool(name="atpool", bufs=3) as atpool,
            tc.tile_pool(name="opool", bufs=3) as opool,
            tc.tile_pool(name="psum_o", bufs=4, space="PSUM") as psum_o,
        ):
            bias_t = singles.tile([P, ns], mybir.dt.float32)
            nc.scalar.dma_start(out=bias_t, in_=BIAS[:, :])

            # ---- one-shot dequant into resident SBUF, natural k-order ----
            wsb = wpool.tile([P, NKB, ns], mybir.dt.bfloat16)
            for sb in range(NKB // SB):
                pk = dq.tile([P, SB, ns], mybir.dt.uint8, tag="pk")
                st = dq.tile([P, SB, ns], mybir.dt.bfloat16, tag="st")
                rows = slice(sb * SB * P, (sb + 1) * SB * P)
                nc.scalar.dma_start(
                    out=pk, in_=NIB[rows, :].rearrange("(b p) n -> p b n", p=P))
                nc.scalar.dma_start(
                    out=st, in_=SREP[rows, :].rearrange("(b p) n -> p b n", p=P))
                for j in range(SB):
                    kb = sb * SB + j
                    v = dq.tile([P, ns], mybir.dt.bfloat16, tag="v")
                    nc.vector.tensor_scalar(
                        out=v, in0=pk[:, j, :], scalar1=8, scalar2=None,
                        op0=mybir.AluOpType.subtract)
                    nc.vector.tensor_tensor(
                        out=wsb[:, kb, :], in0=v, in1=st[:, j, :],
                        op=mybir.AluOpType.mult)

            # ---- main loop over 128-row chunks of A ----
            for _rep in range(reps):
                for mc in range(mch):
                    at = atpool.tile([P, NKB, P], mybir.dt.bfloat16)
                    nc.sync.dma_start(out=at, in_=ATB[mc])

                    o_sb = opool.tile([P, ns], mybir.dt.float32)
                    for (n0, nch) in N_CHUNKS:
                        po = psum_o.tile([P, 512], mybir.dt.float32, tag="po")
                        for kb in range(NKB):
                            nc.tensor.matmul(
                                po[:, :nch], lhsT=at[:, kb, :],
                                rhs=wsb[:, kb, n0:n0 + nch],
                                start=(kb == 0), stop=(kb == NKB - 1))
                        nc.vector.tensor_tensor(
                            out=o_sb[:, n0:n0 + nch], in0=po[:, :nch],
                            in1=bias_t[:, n0:n0 + nch], op=mybir.AluOpType.add)
                    nc.scalar.dma_start(out=OUT[mc * P:(mc + 1) * P, :], in_=o_sb)

    nc.finalize()
    return nc


_NC_CACHE = {}


def _get_nc():
    if "nc" not in _NC_CACHE:
        _NC_CACHE["nc"] = build_nc()
    return _NC_CACHE["nc"]


def _prep_a(A):
    """A [M, K] f32 -> blocked bf16 A^T tiles [MCH, P(k), NKB, P(m)]."""
    ab = np.asarray(A, dtype=np.float32).astype(ml_dtypes.bfloat16)
    x = ab.reshape(MCH, P, NKB, P)           # [mc, m, kb, p]
    return np.ascontiguousarray(np.transpose(x, (0, 3, 2, 1)))


def shard_inputs(A, weight, weight_scale, bias):
    atb = _prep_a(A)
    wq = np.asarray(weight, dtype=np.uint8)
    ws = np.asarray(weight_scale, dtype=np.float32)
    bs = np.asarray(bias, dtype=np.float32)

    # unpack nibbles to natural k-order: row 2i = low nibble, row 2i+1 = high
    nib = np.empty((K, N), dtype=np.uint8)
    nib[0::2] = wq & 15
    nib[1::2] = wq >> 4
    srep = np.repeat(ws, K // ws.shape[0], axis=0).astype(ml_dtypes.bfloat16)

    in_maps = []
    for c in range(NCORES):
        sl = slice(c * NS, (c + 1) * NS)
        in_maps.append({
            "atb": atb,
            "nib": np.ascontiguousarray(nib[:, sl]),
            "srep": np.ascontiguousarray(srep[:, sl]),
            # partition-replicated so the device DMA is a plain 2D copy
            "bias": np.ascontiguousarray(np.broadcast_to(bs[sl], (P, NS))),
        })
    return in_maps


def run(inputs, trace=False, **kw):
    nc = _get_nc()
    in_maps = shard_inputs(**inputs)
    res = run_bass_kernel_spmd(nc, in_maps, core_ids=list(range(NCORES)), trace=trace, **kw)
    out = np.concatenate([res.results[c]["out"] for c in range(NCORES)], axis=1)
    return out, res


def kernel(A, weight, weight_scale, bias):
    out, _ = run(dict(A=A, weight=weight, weight_scale=weight_scale, bias=bias))
    return out


# revision 9
# speedup vs baseline: 1.2972x; 1.0169x over previous
"""Trainium2 Bass kernel for: out = A @ dequant_int4(weight, weight_scale) + bias.

Problem shapes (fp32 A, packed-int4 weight):
    A            [8192, 4096] f32
    weight       [2048, 11008] u8   (two int4 nibbles per byte along K;
                                     row 2i = low nibble, row 2i+1 = high nibble)
    weight_scale [128, 11008] f32   (per-group scale, group_size=32 along K)
    bias         [11008] f32
    out          [8192, 11008] f32

Sharding: tensor-parallel along out_features N across 8 NeuronCores.
Each core gets the full A, a 1376-wide column slice of weight/scale/bias and
computes its [8192, 1376] output slice; the host concatenates slices.

Host prep (layout/dtype only): A is cast to bf16 and re-laid-out into
k-major transposed tiles atb[mc, p, kb, m] = A[128*mc + m, 128*kb + p] so
each 128-row chunk's A^T tile image is one contiguous 1 MB DMA; packed int4
bytes are unpacked to one signed int8 per k (value - 8, i.e. int4 bias-8
decode to two's complement) in natural k-order [4096, N]; scales are
repeated 32x to per-k rows (bf16); bias replicated to [128, ns].  The scale
multiply, GEMM and bias add happen on device.

Per-core kernel strategy:
  - Dequant once into a resident SBUF buffer wsb[p, kb, n] (natural k-order:
    k = 128*kb + p) with one DVE op per element: int8 * bf16-scale -> bf16.
  - Per 128-row chunk of A: one contiguous DMA loads at[p, kb, m]; the PE
    runs pure matmuls: 3 n-chunks x 32 k-blocks accumulating in PSUM.  Bias
    is added during the PSUM->SBUF eviction on the DVE.
  - A-tile loads ride the SP HWDGE ring (nc.sync); most other DMAs ride the
    ACT ring (nc.scalar) so the two FIFOs drain in parallel.
  - A few dummy matmuls pinned to the tail of dequant warm the PE's HAM
    clock gate before the main matmul stream starts.
"""

import numpy as np
import ml_dtypes

import concourse.bacc as bacc
import concourse.tile as tile
from concourse import mybir
from concourse.bass_utils import run_bass_kernel_spmd

M, K, N = 8192, 4096, 11008
NCORES = 8
NS = N // NCORES  # 1376 out-features per core
P = 128
NKB = K // P      # 32 k-blocks
MCH = M // P      # 64 m-chunks
SB = 4            # k-blocks per dequant super-DMA
N_CHUNKS = [(0, 512), (512, 512), (1024, 352)]


def build_nc(m=M, ns=NS, reps=1, debug=False):
    """Build the per-core Bass program (identical on all cores)."""
    mch = m // P

    nc = bacc.Bacc(None, target_bir_lowering=False, debug=debug)
    ATB = nc.dram_tensor("atb", [mch, P, NKB, P], mybir.dt.bfloat16, kind="ExternalInput")
    NIB = nc.dram_tensor("nib", [K, ns], mybir.dt.int8, kind="ExternalInput")
    SREP = nc.dram_tensor("srep", [K, ns], mybir.dt.bfloat16, kind="ExternalInput")
    BIAS = nc.dram_tensor("bias", [P, ns], mybir.dt.float32, kind="ExternalInput")
    OUT = nc.dram_tensor("out", [m, ns], mybir.dt.float32, kind="ExternalOutput")

    with tile.TileContext(nc) as tc:
        with (
            tc.tile_pool(name="singles", bufs=1) as singles,
            tc.tile_pool(name="wpool", bufs=1) as wpool,
            tc.tile_pool(name="dq", bufs=2) as dq,
            tc.tile_pool(name="atpool", bufs=4) as atpool,
            tc.tile_pool(name="opool", bufs=3) as opool,
            tc.tile_pool(name="psum_o", bufs=6, space="PSUM") as psum_o,
        ):
            bias_t = singles.tile([P, ns], mybir.dt.float32)
            nc.scalar.dma_start(out=bias_t, in_=BIAS[:, :])

            # ---- one-shot dequant into resident SBUF, natural k-order ----
            wsb = wpool.tile([P, NKB, ns], mybir.dt.bfloat16)
            for sb in range(NKB // SB):
                pk = dq.tile([P, SB, ns], mybir.dt.int8, tag="pk")
                st = dq.tile([P, SB, ns], mybir.dt.bfloat16, tag="st")
                rows = slice(sb * SB * P, (sb + 1) * SB * P)
                nc.scalar.dma_start(
                    out=pk, in_=NIB[rows, :].rearrange("(b p) n -> p b n", p=P))
                nc.sync.dma_start(
                    out=st, in_=SREP[rows, :].rearrange("(b p) n -> p b n", p=P))
                for j in range(SB):
                    kb = sb * SB + j
                    nc.vector.tensor_tensor(
                        out=wsb[:, kb, :], in0=pk[:, j, :], in1=st[:, j, :],
                        op=mybir.AluOpType.mult)

            # HAM warmup: dummy matmuls pinned (via wsb reads) to the tail of
            # dequant so the PE is at full clock when the real stream begins.
            pwarm = psum_o.tile([P, 512], mybir.dt.float32, tag="po")
            for kb in range(NKB - 16, NKB):
                nc.tensor.matmul(
                    pwarm, lhsT=wsb[:, kb, 0:P], rhs=wsb[:, kb, 0:512],
                    start=(kb == NKB - 16), stop=(kb == NKB - 1))

            # ---- main loop over 128-row chunks of A ----
            for _rep in range(reps):
                for mc in range(mch):
                    at = atpool.tile([P, NKB, P], mybir.dt.bfloat16)
                    nc.sync.dma_start(out=at, in_=ATB[mc])

                    o_sb = opool.tile([P, ns], mybir.dt.float32)
                    for (n0, nch) in N_CHUNKS:
                        po = psum_o.tile([P, 512], mybir.dt.float32, tag="po")
                        for kb in range(NKB):
                            nc.tensor.matmul(
                                po[:, :nch], lhsT=at[:, kb, :],
                                rhs=wsb[:, kb, n0:n0 + nch],
                                start=(kb == 0), stop=(kb == NKB - 1))
                        nc.vector.tensor_tensor(
                            out=o_sb[:, n0:n0 + nch], in0=po[:, :nch],
                            in1=bias_t[:, n0:n0 + nch], op=mybir.AluOpType.add)
                    nc.scalar.dma_start(out=OUT[mc * P:(mc + 1) * P, :], in_=o_sb)

    nc.finalize()
    return nc


_NC_CACHE = {}


def _get_nc():
    if "nc" not in _NC_CACHE:
        _NC_CACHE["nc"] = build_nc()
    return _NC_CACHE["nc"]


def _prep_a(A):
    """A [M, K] f32 -> blocked bf16 A^T tiles [MCH, P(k), NKB, P(m)]."""
    ab = np.asarray(A, dtype=np.float32).astype(ml_dtypes.bfloat16)
    x = ab.reshape(MCH, P, NKB, P)           # [mc, m, kb, p]
    return np.ascontiguousarray(np.transpose(x, (0, 3, 2, 1)))


def shard_inputs(A, weight, weight_scale, bias):
    atb = _prep_a(A)
    wq = np.asarray(weight, dtype=np.uint8)
    ws = np.asarray(weight_scale, dtype=np.float32)
    bs = np.asarray(bias, dtype=np.float32)

    # unpack nibbles to natural k-order (row 2i = low nibble, row 2i+1 =
    # high) and decode bias-8 int4 to two's-complement int8
    nib = np.empty((K, N), dtype=np.int8)
    nib[0::2] = (wq & 15).astype(np.int8) - 8
    nib[1::2] = (wq >> 4).astype(np.int8) - 8
    srep = np.repeat(ws, K // ws.shape[0], axis=0).astype(ml_dtypes.bfloat16)

    in_maps = []
    for c in range(NCORES):
        sl = slice(c * NS, (c + 1) * NS)
        in_maps.append({
            "atb": atb,
            "nib": np.ascontiguousarray(nib[:, sl]),
            "srep": np.ascontiguousarray(srep[:, sl]),
            # partition-replicated so the device DMA is a plain 2D copy
            "bias": np.ascontiguousarray(np.broadcast_to(bs[sl], (P, NS))),
        })
    return in_maps


def run(inputs, trace=False, **kw):
    nc = _get_nc()
    in_maps = shard_inputs(**inputs)
    res = run_bass_kernel_spmd(nc, in_maps, core_ids=list(range(NCORES)), trace=trace, **kw)
    out = np.concatenate([res.results[c]["out"] for c in range(NCORES)], axis=1)
    return out, res


def kernel(A, weight, weight_scale, bias):
    out, _ = run(dict(A=A, weight=weight, weight_scale=weight_scale, bias=bias))
    return out
